# revision 27
# baseline (speedup 1.0000x reference)
"""Trainium2 Bass kernel: ragged mean-pool over [1, len_i] + Linear->tanh->Linear head.

Strategy (pure data parallel over batch, 8 NeuronCores):
  * Host: balance the 256 samples across 8 cores (32 each) by row count (LPT),
    gather the rows hidden_states[b, 1:len_b+1, :] into a dense per-core pack
    of 128-row "slices" (row j -> slice j//128, partition j%128), and encode
    rows in fp8 e3m4 with error-feedback quantization chained along the
    partition axis inside each slice -- the carries cancel in the device-side
    per-sample sums (~0.6% pooled error vs ~1.1% plain RNE).
  * The 32 samples per core are split into an EARLY group (24 samples, packed
    first) and a LATE group (8 samples totalling ~2048 rows, packed last, 16
    slices): the EARLY group's head (scale -> transpose -> dense -> tanh ->
    classifier -> store) runs WHILE the LATE group's rows still stream, so
    only the small LATE head remains after the last byte lands.
  * Device: stream the packed rows on the sync HWDGE queue in chunks that
    taper to single slices (chunk sems gate the PE and fire a DMA receipt
    round-trip after the last byte). Member/consts/head-weights ride the
    scalar HWDGE queue concurrently. Pooling = PE matmuls with the 0/1
    membership matrix (fp8) as stationary operand, THREE concurrent
    256-column streams on disjoint PE column strips into the group's PSUM
    bank. Mean scale (1/len) rides one wide DVE copy per group.
  * Host: scatter per-core logits [32, 96] back to the full [256, 96].
"""

import os
from contextlib import ExitStack

import numpy as np
import ml_dtypes

import concourse.bass as bass
import concourse.mybir as mybir
from concourse import bacc, bass_utils

B, S, H, T_OUT = 256, 512, 768, 96
N_CORES = 8
LOCAL_B = B // N_CORES        # 32 samples per core
NE, NL = 24, 8                # EARLY / LATE group sizes
L_SLICES = 16                 # LATE region is always 16 slices (<=2048 rows)
F32 = mybir.dt.float32
F16 = mybir.dt.float16
F8 = mybir.dt.float8e3       # e3m4: 1-3-4, max 15.5, ~1.1% RMS quant err
NP_F8 = ml_dtypes.float8_e3m4

DWT8 = int(os.environ.get("KERNEL_DWT8", "1"))
MODE = f"f8e3-grp{NE}.{NL}-dwt8{DWT8}"

_cache: dict = {}
last_results = None  # BassKernelResults of the most recent run (for test.py)


def _chunk_split(n_slices: int) -> tuple:
    """Split n_slices into DMA chunks: moderate middles (the PE waits out a
    ~1-2.5us completion-receipt lag at each chunk sem) tapering to single
    slices at the end so the final exposure is one slice's receipt."""
    tail = [4, 3, 2, 2, 1, 1]
    if n_slices <= 14:
        out, left = [], n_slices
        for c in [4, 3, 2, 2, 1, 1, 1]:
            if left <= 0:
                break
            out.append(min(c, left))
            left -= out[-1]
        while left > 0:
            out.append(1)
            left -= 1
        return tuple(out)
    head = [8]
    rest = n_slices - sum(head) - sum(tail)
    n_mid = max(1, -(-rest // 6))
    base, extra = divmod(rest, n_mid)
    return tuple(
        head + [base + (1 if i < extra else 0) for i in range(n_mid)] + tail
    )


def _build_program(n_slices: int, dwt8: int = 0, bias0: int = 0) -> bass.Bass:
    assert n_slices > L_SLICES + 2
    ka = n_slices - L_SLICES          # first LATE slice
    chunks = _chunk_split(n_slices)
    n_chunks = len(chunks)
    chunk_start = np.cumsum([0] + list(chunks))[:-1]
    start_to_chunk = {int(s): i for i, s in enumerate(chunk_start)}

    nc = bacc.Bacc(enable_partition_id=False, monotonic_sem_count=0)

    hsb_d = nc.declare_dram_parameter("hsb", [128, n_slices * H], F8, isOutput=False)
    member_d = nc.declare_dram_parameter(
        "member", [128, n_slices * LOCAL_B], F8, isOutput=False
    )
    WDT = F8 if dwt8 else F16
    dwT_d = nc.declare_dram_parameter("dwT", [128, 6 * H], WDT, isOutput=False)
    cwT_d = nc.declare_dram_parameter("cwT", [128, 6 * T_OUT], F16, isOutput=False)
    identd_d = nc.declare_dram_parameter("identd", [96, LOCAL_B], F16, isOutput=False)
    smalls_d = nc.declare_dram_parameter("smalls", [128, 8], F32, isOutput=False)
    clsb_d = nc.declare_dram_parameter("clsb", [40, T_OUT], F32, isOutput=False)
    out_d = nc.declare_dram_parameter("out", [LOCAL_B, T_OUT], F16, isOutput=True)

    with ExitStack() as ctx:
        hs_sb = ctx.enter_context(nc.sbuf_tensor([128, n_slices * H], F8))
        member_t = ctx.enter_context(nc.sbuf_tensor([128, n_slices * LOCAL_B], F8))
        dwT_t = ctx.enter_context(nc.sbuf_tensor([128, 6 * H], WDT))
        cwT_t = ctx.enter_context(nc.sbuf_tensor([128, 6 * T_OUT], F16))
        identd_t = ctx.enter_context(nc.sbuf_tensor([96, LOCAL_B], F16))
        smalls_t = ctx.enter_context(nc.sbuf_tensor([128, 8], F32))
        clsb_t = ctx.enter_context(nc.sbuf_tensor([40, T_OUT], F32))
        # pooled_sb [96, 512]: strip s at partitions 32s:32s+32; EARLY scaled
        # pool at cols 0:256, LATE at cols 256:512.
        pooled_sb = ctx.enter_context(nc.sbuf_tensor([96, 512], F16))
        # pooledT/hT: EARLY cols 0:144 (24 per hidden chunk), LATE 144:192.
        pooledT_sb = ctx.enter_context(nc.sbuf_tensor([128, 192], F16))
        hT_sb = ctx.enter_context(nc.sbuf_tensor([128, 192], F16))
        logits_sb = ctx.enter_context(nc.sbuf_tensor([40, T_OUT], F16))
        warm_sb = ctx.enter_context(nc.sbuf_tensor([128, 512], F8))
        scratch_sb = ctx.enter_context(nc.sbuf_tensor([128, 8], F32))

        # PSUM (8 banks): pooled_E, pooled_L, tp0, tp1, hps0, hps1, lps, fill
        pooled_E = ctx.enter_context(nc.psum_tensor("poolE", [96, 512], F32))
        pooled_L = ctx.enter_context(nc.psum_tensor("poolL", [96, 512], F32))
        tp = [
            ctx.enter_context(nc.psum_tensor(f"tp{i}", [128, 512], F16))
            for i in range(2)
        ]
        hps = [
            ctx.enter_context(nc.psum_tensor(f"hps{i}", [128, 512], F32))
            for i in range(2)
        ]
        lps = ctx.enter_context(nc.psum_tensor("lps", [40, 512], F32))
        fill = ctx.enter_context(nc.psum_tensor("fill", [128, 512], F32))

        db6_ap = smalls_t[:, 0:6]

        s_member = nc.alloc_semaphore("s_member")
        s_chunk = [nc.alloc_semaphore(f"s_chunk{i}") for i in range(n_chunks)]
        s_smalls = nc.alloc_semaphore("s_smalls")
        s_dwTa = nc.alloc_semaphore("s_dwTa")
        s_dwTb = nc.alloc_semaphore("s_dwTb")
        s_cwT = nc.alloc_semaphore("s_cwT")
        s_warm = nc.alloc_semaphore("s_warm")
        s_poolE = nc.alloc_semaphore("s_poolE")
        s_poolL = nc.alloc_semaphore("s_poolL")
        s_scE = nc.alloc_semaphore("s_scE")
        s_scL = nc.alloc_semaphore("s_scL")
        s_trE = nc.alloc_semaphore("s_trE")
        s_trL = nc.alloc_semaphore("s_trL")
        s_ptcE = nc.alloc_semaphore("s_ptcE")
        s_ptcL = nc.alloc_semaphore("s_ptcL")
        s_headE = nc.alloc_semaphore("s_headE")
        s_headL = nc.alloc_semaphore("s_headL")
        s_tanhE = nc.alloc_semaphore("s_tanhE")
        s_tanhL = nc.alloc_semaphore("s_tanhL")
        s_clsE = nc.alloc_semaphore("s_clsE")
        s_clsL = nc.alloc_semaphore("s_clsL")
        s_logE = nc.alloc_semaphore("s_logE")
        s_logL = nc.alloc_semaphore("s_logL")
        s_out = nc.alloc_semaphore("s_out")

        # ---- PE head helpers (emitted per group) ----
        def pe_transposes(grp):
            # 6 PE transposes [W, 128] -> tp[c%2][:, :W]; copies chase on
            # DVE (even c) / ACT (odd c) into pooledT_sb.
            if grp == "E":
                nc.tensor.wait_ge(s_scE, 1)
            else:
                nc.tensor.wait_ge(s_scL, 1)
            for c in range(6):
                s = c // 2
                if c >= 2:
                    nc.tensor.wait_ge(s_ptcE if grp == "E" else s_ptcL, c - 1)
                if grp == "E":
                    src = pooled_sb[
                        32 * s : 32 * s + NE,
                        (c % 2) * 128 : (c % 2) * 128 + 128,
                    ]
                    ident = identd_t[32 * s : 32 * s + NE, 0:NE]
                    w = NE
                else:
                    # LATE samples sit at member cols 0:8 of their slices,
                    # so their pooled rows are at partitions 32s:32s+8
                    # (PE operands need 32-aligned base partitions).
                    src = pooled_sb[
                        32 * s : 32 * s + NL,
                        256 + (c % 2) * 128 : 256 + (c % 2) * 128 + 128,
                    ]
                    ident = identd_t[32 * s : 32 * s + NL, 0:NL]
                    w = NL
                nc.tensor.transpose(tp[c % 2][:, :w], src, ident).then_inc(
                    s_trE if grp == "E" else s_trL, 1
                )

        def pe_dense(grp):
            if grp == "E":
                nc.tensor.wait_ge(s_ptcE, 6)
                nc.tensor.wait_ge(s_dwTa, 16)
                nc.tensor.wait_ge(s_dwTb, 16)
            else:
                nc.tensor.wait_ge(s_ptcL, 6)
                # hps banks were read by the EARLY tanhs; make sure those
                # ACT reads retired before PE writes the banks again.
                nc.tensor.wait_ge(s_tanhE, 2 if bias0 else 6)
            for jg in range(6):
                if grp == "E":
                    out_ap = hps[jg // 3][:, (jg % 3) * NE : (jg % 3 + 1) * NE]
                    mv = pooledT_sb
                    mvc = lambda c: mv[:, c * NE : (c + 1) * NE]
                else:
                    out_ap = hps[0][:, 96 + jg * NL : 96 + (jg + 1) * NL]
                    mvc = lambda c: pooledT_sb[
                        :, 144 + c * NL : 144 + (c + 1) * NL
                    ]
                for c in range(6):
                    mm = nc.tensor.matmul(
                        out_ap,
                        dwT_t[:, jg * H + c * 128 : jg * H + (c + 1) * 128],
                        mvc(c),
                        start=(c == 0), stop=(c == 5),
                    )
                mm.then_inc(s_headE if grp == "E" else s_headL, 1)

        def pe_cls(grp):
            nc.tensor.wait_ge(s_cwT, 16)
            if grp == "L":
                # lps was read by the EARLY logits add (DVE).
                nc.tensor.wait_ge(s_logE, 1)
            for jg in range(6):
                if grp == "E":
                    if bias0:
                        nc.tensor.wait_ge(s_tanhE, 1 if jg < 3 else 2)
                    else:
                        nc.tensor.wait_ge(s_tanhE, jg + 1)
                    lhsT = hT_sb[:, jg * NE : (jg + 1) * NE]
                    out_ap = lps[0:NE, :T_OUT]
                else:
                    if bias0:
                        nc.tensor.wait_ge(s_tanhL, 1)
                    else:
                        nc.tensor.wait_ge(s_tanhL, jg + 1)
                    lhsT = hT_sb[:, 144 + jg * NL : 144 + (jg + 1) * NL]
                    out_ap = lps[32 : 32 + NL, :T_OUT]
                mm = nc.tensor.matmul(
                    out_ap,
                    lhsT,
                    cwT_t[:, jg * T_OUT : (jg + 1) * T_OUT],
                    start=(jg == 0), stop=(jg == 5),
                )
            mm.then_inc(s_clsE if grp == "E" else s_clsL, 1)

        with nc.Block(no_gpsimd_drain=True) as block:

            @block.gpsimd
            def _(gpsimd):
                nc.gpsimd.memset(warm_sb[:], 0.0).then_inc(s_warm, 1)

            @block.sync
            def _(sync):
                # Sync HWDGE queue: the hs stream only (a second queue of
                # chunks made SDMA engine 15 a laggard; keep chunks here).
                for ci, (cs, cn) in enumerate(zip(chunk_start, chunks)):
                    sync.dma_start(
                        out=hs_sb[:, cs * H : (cs + cn) * H],
                        in_=hsb_d[:, cs * H : (cs + cn) * H],
                    ).then_inc(s_chunk[ci], 16)
                sync.wait_ge(s_logE, 1)
                sync.dma_start(
                    out=out_d[0:NE, :], in_=logits_sb[0:NE, :]
                ).then_inc(s_out, 16)
                sync.wait_ge(s_logL, 1)
                sync.dma_start(
                    out=out_d[NE:LOCAL_B, :], in_=logits_sb[32 : 32 + NL, :]
                ).then_inc(s_out, 16)
                # Completion wait: teardown's dma_reset must not overlap the
                # in-flight store.
                sync.wait_ge(s_out, 32)

            @block.tensor
            def _(tensor):
                # Warmup fillers: ungate the PE clock (HAM) until chunk 0.
                tensor.wait_ge(s_warm, 1)
                for _ in range(20):
                    nc.tensor.matmul(
                        fill[:, :512], warm_sb[:, :128], warm_sb[:, :512],
                        start=True, stop=True,
                    )

                tensor.wait_ge(s_member, 16)
                for k in range(n_slices):
                    ci = start_to_chunk.get(k)
                    if ci is not None:
                        if 1 <= ci < n_chunks - 6:
                            # bridge fillers: keep HAM busy across the chunk
                            # sem's receipt lag; none on the tail chunks
                            # where the PE drains backlog.
                            for _ in range(4):
                                nc.tensor.matmul(
                                    fill[:, :256], warm_sb[:, :128],
                                    warm_sb[:, :256], start=True, stop=True,
                                )
                        tensor.wait_ge(s_chunk[ci], 16)
                    # EARLY head interleaves into the LATE pooling stretch.
                    if k == ka + 2:
                        pe_transposes("E")
                    elif k == ka + 6:
                        pe_dense("E")
                    elif k == ka + 10:
                        pe_cls("E")
                    lhsT = member_t[:, k * LOCAL_B : (k + 1) * LOCAL_B]
                    rs = k * H
                    grp_ps = pooled_E if k < ka else pooled_L
                    first = k == 0 or k == ka
                    last = k == ka - 1 or k == n_slices - 1
                    for s in range(3):
                        mm = nc.tensor.matmul(
                            grp_ps[32 * s : 32 * (s + 1), :256],
                            lhsT,
                            hs_sb[:, rs + 256 * s : rs + 256 * (s + 1)],
                            start=first, stop=last,
                        )
                        if last:
                            mm.then_inc(s_poolE if k < ka else s_poolL, 1)

                # LATE head (tail)
                pe_transposes("L")
                pe_dense("L")
                pe_cls("L")

            @block.vector
            def _(vector):
                # EARLY mean scale: one wide op across all three strips.
                vector.wait_ge(s_smalls, 48)
                vector.wait_ge(s_poolE, 3)
                nc.vector.tensor_scalar_mul(
                    pooled_sb[0:96, 0:256],
                    pooled_E[0:96, :256],
                    smalls_t[0:96, 6:7],
                ).then_inc(s_scE, 1)
                for c in (0, 2, 4):
                    vector.wait_ge(s_trE, c + 1)
                    nc.vector.tensor_copy(
                        pooledT_sb[:, c * NE : (c + 1) * NE],
                        tp[c % 2][:, :NE],
                    ).then_inc(s_ptcE, 1)
                vector.wait_ge(s_clsE, 1)
                nc.vector.tensor_add(
                    logits_sb[0:NE, :], lps[0:NE, :T_OUT], clsb_t[0:NE, :]
                ).then_inc(s_logE, 1)
                # LATE
                vector.wait_ge(s_poolL, 3)
                nc.vector.tensor_scalar_mul(
                    pooled_sb[0:96, 256:512],
                    pooled_L[0:96, :256],
                    smalls_t[0:96, 7:8],
                ).then_inc(s_scL, 1)
                for c in (0, 2, 4):
                    vector.wait_ge(s_trL, c + 1)
                    nc.vector.tensor_copy(
                        pooledT_sb[:, 144 + c * NL : 144 + (c + 1) * NL],
                        tp[c % 2][:, :NL],
                    ).then_inc(s_ptcL, 1)
                vector.wait_ge(s_clsL, 1)
                # clsb rows are all identical (broadcast cls_b), so rows
                # 32:40 are partition-aligned with the LATE lps region.
                nc.vector.tensor_add(
                    logits_sb[32 : 32 + NL, :],
                    lps[32 : 32 + NL, :T_OUT],
                    clsb_t[32 : 32 + NL, :],
                ).then_inc(s_logL, 1)

            @block.scalar
            def _(scalar):
                # ACT HWDGE queue: member + consts + head weights.
                scalar.dma_start(out=member_t[:], in_=member_d[:]).then_inc(
                    s_member, 16
                )
                scalar.dma_start(out=smalls_t[:], in_=smalls_d[:]).then_inc(
                    s_smalls, 16
                )
                scalar.dma_start(out=identd_t[:], in_=identd_d[:]).then_inc(
                    s_smalls, 16
                )
                scalar.dma_start(out=clsb_t[:], in_=clsb_d[:]).then_inc(
                    s_smalls, 16
                )
                scalar.dma_start(
                    out=dwT_t[:, : 3 * H], in_=dwT_d[:, : 3 * H]
                ).then_inc(s_dwTa, 16)
                scalar.dma_start(
                    out=dwT_t[:, 3 * H :], in_=dwT_d[:, 3 * H :]
                ).then_inc(s_dwTb, 16)
                scalar.dma_start(out=cwT_t[:], in_=cwT_d[:]).then_inc(s_cwT, 16)
                # Dummy tanh: pulls the lazy ACT_TABLE_LOAD (~1.3us) forward.
                nc.scalar.activation(
                    scratch_sb[:, 0:1], warm_sb[:, 0:1],
                    mybir.ActivationFunctionType.Tanh,
                )

                def copies_tanh(grp):
                    s_tr = s_trE if grp == "E" else s_trL
                    for c in (1, 3, 5):
                        scalar.wait_ge(s_tr, c + 1)
                        if grp == "E":
                            dst = pooledT_sb[:, c * NE : (c + 1) * NE]
                            w = NE
                        else:
                            dst = pooledT_sb[:, 144 + c * NL : 144 + (c + 1) * NL]
                            w = NL
                        nc.scalar.activation(
                            dst, tp[c % 2][:, :w],
                            mybir.ActivationFunctionType.Copy,
                        ).then_inc(s_ptcE if grp == "E" else s_ptcL, 1)
                    sc = (1.0 / 64.0) if dwt8 else 1.0
                    if grp == "E":
                        if bias0:
                            for half in range(2):
                                scalar.wait_ge(s_headE, 3 * (half + 1))
                                nc.scalar.activation(
                                    hT_sb[:, 72 * half : 72 * (half + 1)],
                                    hps[half][:, : 3 * NE],
                                    mybir.ActivationFunctionType.Tanh,
                                    scale=sc,
                                ).then_inc(s_tanhE, 1)
                        else:
                            for jg in range(6):
                                scalar.wait_ge(s_headE, jg + 1)
                                nc.scalar.activation(
                                    hT_sb[:, jg * NE : (jg + 1) * NE],
                                    hps[jg // 3][
                                        :, (jg % 3) * NE : (jg % 3 + 1) * NE
                                    ],
                                    mybir.ActivationFunctionType.Tanh,
                                    bias=db6_ap[:, jg : jg + 1],
                                    scale=sc,
                                ).then_inc(s_tanhE, 1)
                    else:
                        if bias0:
                            scalar.wait_ge(s_headL, 6)
                            nc.scalar.activation(
                                hT_sb[:, 144:192],
                                hps[0][:, 96:144],
                                mybir.ActivationFunctionType.Tanh,
                                scale=sc,
                            ).then_inc(s_tanhL, 1)
                        else:
                            for jg in range(6):
                                scalar.wait_ge(s_headL, jg + 1)
                                nc.scalar.activation(
                                    hT_sb[:, 144 + jg * NL : 144 + (jg + 1) * NL],
                                    hps[0][:, 96 + jg * NL : 96 + (jg + 1) * NL],
                                    mybir.ActivationFunctionType.Tanh,
                                    bias=db6_ap[:, jg : jg + 1],
                                    scale=sc,
                                ).then_inc(s_tanhL, 1)

                copies_tanh("E")
                copies_tanh("L")

    nc.compile()
    return nc


def _ef_quantize(packed: np.ndarray, n_slices: int) -> np.ndarray:
    """Error-feedback quantization to fp8 e3m4, carried along the partition
    axis within each 128-row slice so each sample's device-side sum error
    collapses to its few chain-boundary carries."""
    arr = packed.reshape(n_slices, 128, H)
    q8 = np.empty((n_slices, 128, H), NP_F8)
    c = np.zeros((n_slices, H), np.float32)
    for p in range(128):
        y = arr[:, p, :] + c
        q = y.astype(NP_F8)
        c = y - q.astype(np.float32)
        q8[:, p, :] = q
    return q8


def _split_groups(lens_c: np.ndarray):
    """Pick NL samples for the LATE group with total rows <= L_SLICES*128,
    as close to it as possible (their stream time hides the EARLY head)."""
    cap = L_SLICES * 128
    order = np.argsort(lens_c, kind="stable")          # ascending
    late = list(order[:NL])                            # start: NL smallest
    rest = list(order[NL:])
    lsum = int(lens_c[late].sum())
    improved = True
    while improved:
        improved = False
        for i in range(len(late)):
            for j in range(len(rest)):
                d = int(lens_c[rest[j]]) - int(lens_c[late[i]])
                if d > 0 and lsum + d <= cap:
                    late[i], rest[j] = rest[j], late[i]
                    lsum += d
                    improved = True
                    break
            if improved:
                break
    early = [i for i in range(len(lens_c)) if i not in set(late)]
    return early, late


def kernel(hidden_states, pivot_len_list, dense_w, dense_b, cls_w, cls_b):
    global last_results
    hs = np.ascontiguousarray(np.asarray(hidden_states, dtype=np.float32))
    lens = np.asarray(pivot_len_list).astype(np.int64)
    dense_w = np.asarray(dense_w, dtype=np.float32)
    dense_b = np.asarray(dense_b, dtype=np.float32)
    cls_w = np.asarray(cls_w, dtype=np.float32)
    cls_b = np.asarray(cls_b, dtype=np.float32)
    assert hs.shape == (B, S, H), hs.shape
    assert lens.shape == (B,), lens.shape

    # ---- assign samples to cores: greedy LPT with a hard 32-per-core cap
    order = np.argsort(-lens, kind="stable")
    core_samples = [[] for _ in range(N_CORES)]
    load = np.zeros(N_CORES, dtype=np.int64)
    for b in order:
        open_cores = [c for c in range(N_CORES) if len(core_samples[c]) < LOCAL_B]
        c = min(open_cores, key=lambda c: load[c])
        core_samples[c].append(int(b))
        load[c] += int(lens[b])

    # ---- EARLY/LATE split per core; shared Ka across cores
    core_groups = []
    ka_need = 2
    for c in range(N_CORES):
        lens_c = lens[core_samples[c]]
        early, late = _split_groups(lens_c)
        rows_e = int(lens_c[early].sum())
        ka_need = max(ka_need, -(-rows_e // 128))
        core_groups.append((early, late))
    n_slices = ka_need + L_SLICES
    ka = ka_need

    bias0 = int(np.all(dense_b == 0.0))
    key = (n_slices, DWT8, bias0)
    if key not in _cache:
        _cache[key] = _build_program(n_slices, DWT8, bias0)
    nc = _cache[key]

    # ---- shared (replicated) head tensors
    dwT_host = np.empty((128, 6 * H), np.float32)
    for jg in range(6):
        for cc in range(6):
            dwT_host[:, jg * H + cc * 128 : jg * H + (cc + 1) * 128] = dense_w[
                jg * 128 : (jg + 1) * 128, cc * 128 : (cc + 1) * 128
            ].T
    cwT_host = np.empty((128, 6 * T_OUT), np.float32)
    for jg in range(6):
        cwT_host[:, jg * T_OUT : (jg + 1) * T_OUT] = cls_w[
            :, jg * 128 : (jg + 1) * 128
        ].T
    smalls_base = np.zeros((128, 8), np.float32)
    smalls_base[:, 0:6] = dense_b.reshape(6, 128).T
    clsb_host = np.ascontiguousarray(
        np.broadcast_to(cls_b, (40, T_OUT)).astype(np.float32)
    )
    identd_host = np.zeros((96, LOCAL_B), np.float16)
    for s in range(3):
        identd_host[32 * s : 32 * (s + 1)] = np.eye(LOCAL_B, dtype=np.float16)

    # ---- per-core packing: EARLY rows pad to ka slices, then LATE rows
    hs2 = hs.reshape(B * S, H)
    NR = n_slices * 128
    in_maps = []
    for c in range(N_CORES):
        samples = core_samples[c]
        lens_c = lens[samples]
        early, late = core_groups[c]
        ordered = early + late                  # local order: EARLY then LATE
        samples_ord = [samples[i] for i in ordered]
        lens_ord = lens_c[ordered]

        packed = np.zeros((NR, H), np.float32)
        mem = np.zeros((128, n_slices * LOCAL_B), NP_F8)

        def put(rows_idx, local_bs, base):
            n = rows_idx.size
            packed[base : base + n] = hs2[rows_idx]
            j = base + np.arange(n)
            kq = j // 128
            p = j % 128
            mem[p, kq * LOCAL_B + local_bs] = NP_F8(1.0)

        idx_e = np.concatenate(
            [np.arange(samples[i] * S + 1, samples[i] * S + 1 + lens_c[i])
             for i in early]
        )
        lb_e = np.repeat(np.arange(NE), lens_c[early])
        put(idx_e, lb_e, 0)
        idx_l = np.concatenate(
            [np.arange(samples[i] * S + 1, samples[i] * S + 1 + lens_c[i])
             for i in late]
        )
        lb_l = np.repeat(np.arange(NL), lens_c[late])
        put(idx_l, lb_l, ka * 128)

        q8 = _ef_quantize(packed, n_slices)
        hsb_host = np.ascontiguousarray(
            q8.transpose(1, 0, 2).reshape(128, n_slices * H)
        )

        invl = 1.0 / lens_ord.astype(np.float32)
        smalls_host = smalls_base.copy()
        for s in range(3):
            smalls_host[32 * s : 32 * s + NE, 6] = invl[:NE]
            smalls_host[32 * s : 32 * s + NL, 7] = invl[NE:]

        in_maps.append(
            {
                "hsb": hsb_host,
                "member": mem,
                "dwT": (np.clip(dwT_host * 64.0, -15.5, 15.5).astype(NP_F8)
                        if DWT8 else dwT_host.astype(np.float16)),
                "cwT": cwT_host.astype(np.float16),
                "identd": identd_host,
                "smalls": smalls_host,
                "clsb": clsb_host,
            }
        )
        core_samples[c] = samples_ord

    trace = bool(os.environ.get("KERNEL_TRACE"))
    try:
        res = bass_utils.run_bass_kernel_spmd(
            nc, in_maps, list(range(N_CORES)), trace=trace
        )
    except Exception:
        # Transient NRT device errors clear on retry.
        res = bass_utils.run_bass_kernel_spmd(
            nc, in_maps, list(range(N_CORES)), trace=trace
        )
    last_results = res

    logits = np.zeros((B, T_OUT), np.float32)
    for c in range(N_CORES):
        logits[core_samples[c], :] = res.results[c]["out"].astype(np.float32)
    return logits


# revision 28
# speedup vs baseline: 1.0580x; 1.0580x over previous
"""Trainium2 Bass kernel: ragged mean-pool over [1, len_i] + Linear->tanh->Linear head.

Strategy (pure data parallel over batch, 8 NeuronCores):
  * Host: balance the 256 samples across 8 cores (32 each) by row count (LPT),
    gather the rows hidden_states[b, 1:len_b+1, :] into a dense per-core pack
    of 128-row "slices" (row j -> slice j//128, partition j%128), and encode
    rows in fp8 e3m4 with error-feedback quantization chained along the
    partition axis inside each slice -- the carries cancel in the device-side
    per-sample sums (~0.6% pooled error vs ~1.1% plain RNE).
  * The 32 samples per core are split into an EARLY group (24 samples, packed
    first) and a LATE group (8 samples totalling ~2048 rows, packed last, 16
    slices): the EARLY group's head (scale -> transpose -> dense -> tanh ->
    classifier -> store) runs WHILE the LATE group's rows still stream, so
    only the small LATE head remains after the last byte lands.
  * Device: stream the packed rows on the sync HWDGE queue in chunks that
    taper to single slices (chunk sems gate the PE and fire a DMA receipt
    round-trip after the last byte). Member/consts/head-weights ride the
    scalar HWDGE queue concurrently. Pooling = PE matmuls with the 0/1
    membership matrix (fp8) as stationary operand, THREE concurrent
    256-column streams on disjoint PE column strips into the group's PSUM
    bank. Mean scale (1/len) rides one wide DVE copy per group.
  * Host: scatter per-core logits [32, 96] back to the full [256, 96].
"""

import os
from contextlib import ExitStack

import numpy as np
import ml_dtypes

import concourse.bass as bass
import concourse.mybir as mybir
from concourse import bacc, bass_utils

B, S, H, T_OUT = 256, 512, 768, 96
N_CORES = 8
LOCAL_B = B // N_CORES        # 32 samples per core
NE, NL = 24, 8                # EARLY / LATE group sizes
L_SLICES = 28                 # LATE region slice count (<=3584 rows)
F32 = mybir.dt.float32
F16 = mybir.dt.float16
F8 = mybir.dt.float8e3       # e3m4: 1-3-4, max 15.5, ~1.1% RMS quant err
NP_F8 = ml_dtypes.float8_e3m4

DWT8 = int(os.environ.get("KERNEL_DWT8", "1"))
MODE = f"f8e3-grp{NE}.{NL}-dwt8{DWT8}"

_cache: dict = {}
last_results = None  # BassKernelResults of the most recent run (for test.py)


def _chunk_split(n_slices: int) -> tuple:
    """Split n_slices into DMA chunks: moderate middles (the PE waits out a
    ~1-2.5us completion-receipt lag at each chunk sem) tapering to single
    slices at the end so the final exposure is one slice's receipt."""
    tail = [4, 3, 2, 2, 1, 1]
    if n_slices <= 14:
        out, left = [], n_slices
        for c in [4, 3, 2, 2, 1, 1, 1]:
            if left <= 0:
                break
            out.append(min(c, left))
            left -= out[-1]
        while left > 0:
            out.append(1)
            left -= 1
        return tuple(out)
    head = [8]
    rest = n_slices - sum(head) - sum(tail)
    n_mid = max(1, -(-rest // 6))
    base, extra = divmod(rest, n_mid)
    return tuple(
        head + [base + (1 if i < extra else 0) for i in range(n_mid)] + tail
    )


def _build_program(n_slices: int, dwt8: int = 0, bias0: int = 0) -> bass.Bass:
    assert n_slices > L_SLICES + 2
    ka = n_slices - L_SLICES          # first LATE slice
    chunks = _chunk_split(n_slices)
    n_chunks = len(chunks)
    chunk_start = np.cumsum([0] + list(chunks))[:-1]
    start_to_chunk = {int(s): i for i, s in enumerate(chunk_start)}

    nc = bacc.Bacc(enable_partition_id=False, monotonic_sem_count=0)

    hsb_d = nc.declare_dram_parameter("hsb", [128, n_slices * H], F8, isOutput=False)
    member_d = nc.declare_dram_parameter(
        "member", [128, n_slices * LOCAL_B], F8, isOutput=False
    )
    WDT = F8 if dwt8 else F16
    dwT_d = nc.declare_dram_parameter("dwT", [128, 6 * H], WDT, isOutput=False)
    cwT_d = nc.declare_dram_parameter("cwT", [128, 6 * T_OUT], F16, isOutput=False)
    identd_d = nc.declare_dram_parameter("identd", [96, LOCAL_B], F16, isOutput=False)
    smalls_d = nc.declare_dram_parameter("smalls", [128, 8], F32, isOutput=False)
    clsb_d = nc.declare_dram_parameter("clsb", [40, T_OUT], F32, isOutput=False)
    out_d = nc.declare_dram_parameter("out", [LOCAL_B, T_OUT], F16, isOutput=True)

    with ExitStack() as ctx:
        hs_sb = ctx.enter_context(nc.sbuf_tensor([128, n_slices * H], F8))
        member_t = ctx.enter_context(nc.sbuf_tensor([128, n_slices * LOCAL_B], F8))
        dwT_t = ctx.enter_context(nc.sbuf_tensor([128, 6 * H], WDT))
        cwT_t = ctx.enter_context(nc.sbuf_tensor([128, 6 * T_OUT], F16))
        identd_t = ctx.enter_context(nc.sbuf_tensor([96, LOCAL_B], F16))
        smalls_t = ctx.enter_context(nc.sbuf_tensor([128, 8], F32))
        clsb_t = ctx.enter_context(nc.sbuf_tensor([40, T_OUT], F32))
        # pooled_sb [96, 512]: strip s at partitions 32s:32s+32; EARLY scaled
        # pool at cols 0:256, LATE at cols 256:512.
        pooled_sb = ctx.enter_context(nc.sbuf_tensor([96, 512], F16))
        # pooledT/hT: EARLY cols 0:144 (24 per hidden chunk), LATE 144:192.
        pooledT_sb = ctx.enter_context(nc.sbuf_tensor([128, 192], F16))
        hT_sb = ctx.enter_context(nc.sbuf_tensor([128, 192], F16))
        logits_sb = ctx.enter_context(nc.sbuf_tensor([40, T_OUT], F16))
        warm_sb = ctx.enter_context(nc.sbuf_tensor([128, 512], F8))
        scratch_sb = ctx.enter_context(nc.sbuf_tensor([128, 8], F32))

        # PSUM (8 banks): pooled_E, pooled_L, tp0, tp1, hps0, hps1, lps, fill
        pooled_E = ctx.enter_context(nc.psum_tensor("poolE", [96, 512], F32))
        pooled_L = ctx.enter_context(nc.psum_tensor("poolL", [96, 512], F32))
        tp = [
            ctx.enter_context(nc.psum_tensor(f"tp{i}", [128, 512], F16))
            for i in range(2)
        ]
        hps = [
            ctx.enter_context(nc.psum_tensor(f"hps{i}", [128, 512], F32))
            for i in range(2)
        ]
        lps = ctx.enter_context(nc.psum_tensor("lps", [40, 512], F32))
        fill = ctx.enter_context(nc.psum_tensor("fill", [128, 512], F32))

        db6_ap = smalls_t[:, 0:6]

        s_member = nc.alloc_semaphore("s_member")
        s_chunk = [nc.alloc_semaphore(f"s_chunk{i}") for i in range(n_chunks)]
        s_smalls = nc.alloc_semaphore("s_smalls")
        s_dwTa = nc.alloc_semaphore("s_dwTa")
        s_dwTb = nc.alloc_semaphore("s_dwTb")
        s_cwT = nc.alloc_semaphore("s_cwT")
        s_warm = nc.alloc_semaphore("s_warm")
        s_poolE = nc.alloc_semaphore("s_poolE")
        s_poolL = nc.alloc_semaphore("s_poolL")
        s_scE = nc.alloc_semaphore("s_scE")
        s_scL = nc.alloc_semaphore("s_scL")
        s_trE = nc.alloc_semaphore("s_trE")
        s_trL = nc.alloc_semaphore("s_trL")
        s_ptcE = nc.alloc_semaphore("s_ptcE")
        s_ptcL = nc.alloc_semaphore("s_ptcL")
        s_headE = nc.alloc_semaphore("s_headE")
        s_headL = nc.alloc_semaphore("s_headL")
        s_tanhE = nc.alloc_semaphore("s_tanhE")
        s_tanhL = nc.alloc_semaphore("s_tanhL")
        s_clsE = nc.alloc_semaphore("s_clsE")
        s_clsL = nc.alloc_semaphore("s_clsL")
        s_logE = nc.alloc_semaphore("s_logE")
        s_logL = nc.alloc_semaphore("s_logL")
        s_out = nc.alloc_semaphore("s_out")

        # ---- PE head helpers (emitted per group) ----
        def pe_transposes(grp):
            # 6 PE transposes [W, 128] -> tp[c%2][:, :W]; copies chase on
            # DVE (even c) / ACT (odd c) into pooledT_sb.
            if grp == "E":
                nc.tensor.wait_ge(s_scE, 1)
            else:
                nc.tensor.wait_ge(s_scL, 1)
            for c in range(6):
                s = c // 2
                if c >= 2:
                    nc.tensor.wait_ge(s_ptcE if grp == "E" else s_ptcL, c - 1)
                if grp == "E":
                    src = pooled_sb[
                        32 * s : 32 * s + NE,
                        (c % 2) * 128 : (c % 2) * 128 + 128,
                    ]
                    ident = identd_t[32 * s : 32 * s + NE, 0:NE]
                    w = NE
                else:
                    # LATE samples sit at member cols 0:8 of their slices,
                    # so their pooled rows are at partitions 32s:32s+8
                    # (PE operands need 32-aligned base partitions).
                    src = pooled_sb[
                        32 * s : 32 * s + NL,
                        256 + (c % 2) * 128 : 256 + (c % 2) * 128 + 128,
                    ]
                    ident = identd_t[32 * s : 32 * s + NL, 0:NL]
                    w = NL
                nc.tensor.transpose(tp[c % 2][:, :w], src, ident).then_inc(
                    s_trE if grp == "E" else s_trL, 1
                )

        def pe_dense(grp):
            if grp == "E":
                nc.tensor.wait_ge(s_ptcE, 6)
                nc.tensor.wait_ge(s_dwTa, 16)
                nc.tensor.wait_ge(s_dwTb, 16)
            else:
                nc.tensor.wait_ge(s_ptcL, 6)
                # hps banks were read by the EARLY tanhs; make sure those
                # ACT reads retired before PE writes the banks again.
                nc.tensor.wait_ge(s_tanhE, 2 if bias0 else 6)
            for jg in range(6):
                if grp == "E":
                    out_ap = hps[jg // 3][:, (jg % 3) * NE : (jg % 3 + 1) * NE]
                    mv = pooledT_sb
                    mvc = lambda c: mv[:, c * NE : (c + 1) * NE]
                else:
                    out_ap = hps[0][:, 96 + jg * NL : 96 + (jg + 1) * NL]
                    mvc = lambda c: pooledT_sb[
                        :, 144 + c * NL : 144 + (c + 1) * NL
                    ]
                for c in range(6):
                    mm = nc.tensor.matmul(
                        out_ap,
                        dwT_t[:, jg * H + c * 128 : jg * H + (c + 1) * 128],
                        mvc(c),
                        start=(c == 0), stop=(c == 5),
                    )
                mm.then_inc(s_headE if grp == "E" else s_headL, 1)

        def pe_cls(grp):
            nc.tensor.wait_ge(s_cwT, 16)
            if grp == "L":
                # lps was read by the EARLY logits add (DVE).
                nc.tensor.wait_ge(s_logE, 1)
            for jg in range(6):
                if grp == "E":
                    if bias0:
                        nc.tensor.wait_ge(s_tanhE, 1 if jg < 3 else 2)
                    else:
                        nc.tensor.wait_ge(s_tanhE, jg + 1)
                    lhsT = hT_sb[:, jg * NE : (jg + 1) * NE]
                    out_ap = lps[0:NE, :T_OUT]
                else:
                    if bias0:
                        nc.tensor.wait_ge(s_tanhL, 1)
                    else:
                        nc.tensor.wait_ge(s_tanhL, jg + 1)
                    lhsT = hT_sb[:, 144 + jg * NL : 144 + (jg + 1) * NL]
                    out_ap = lps[32 : 32 + NL, :T_OUT]
                mm = nc.tensor.matmul(
                    out_ap,
                    lhsT,
                    cwT_t[:, jg * T_OUT : (jg + 1) * T_OUT],
                    start=(jg == 0), stop=(jg == 5),
                )
            mm.then_inc(s_clsE if grp == "E" else s_clsL, 1)

        with nc.Block(no_gpsimd_drain=True) as block:

            @block.gpsimd
            def _(gpsimd):
                nc.gpsimd.memset(warm_sb[:], 0.0).then_inc(s_warm, 1)

            @block.sync
            def _(sync):
                # Sync HWDGE queue: the hs stream only (a second queue of
                # chunks made SDMA engine 15 a laggard; keep chunks here).
                for ci, (cs, cn) in enumerate(zip(chunk_start, chunks)):
                    sync.dma_start(
                        out=hs_sb[:, cs * H : (cs + cn) * H],
                        in_=hsb_d[:, cs * H : (cs + cn) * H],
                    ).then_inc(s_chunk[ci], 16)
                sync.wait_ge(s_logE, 1)
                sync.dma_start(
                    out=out_d[0:NE, :], in_=logits_sb[0:NE, :]
                ).then_inc(s_out, 16)
                sync.wait_ge(s_logL, 1)
                sync.dma_start(
                    out=out_d[NE:LOCAL_B, :], in_=logits_sb[32 : 32 + NL, :]
                ).then_inc(s_out, 16)
                # Completion wait: teardown's dma_reset must not overlap the
                # in-flight store.
                sync.wait_ge(s_out, 32)

            @block.tensor
            def _(tensor):
                # Warmup fillers: ungate the PE clock (HAM) until chunk 0.
                tensor.wait_ge(s_warm, 1)
                for _ in range(20):
                    nc.tensor.matmul(
                        fill[:, :512], warm_sb[:, :128], warm_sb[:, :512],
                        start=True, stop=True,
                    )

                tensor.wait_ge(s_member, 16)
                for k in range(n_slices):
                    ci = start_to_chunk.get(k)
                    if ci is not None:
                        if 1 <= ci < n_chunks - 6:
                            # bridge fillers: keep HAM busy across the chunk
                            # sem's receipt lag; none on the tail chunks
                            # where the PE drains backlog.
                            for _ in range(4):
                                nc.tensor.matmul(
                                    fill[:, :256], warm_sb[:, :128],
                                    warm_sb[:, :256], start=True, stop=True,
                                )
                        tensor.wait_ge(s_chunk[ci], 16)
                    # EARLY head interleaves into the LATE pooling stretch.
                    if k == ka + 2:
                        pe_transposes("E")
                    elif k == ka + 8:
                        pe_dense("E")
                    elif k == ka + 14:
                        pe_cls("E")
                    lhsT = member_t[:, k * LOCAL_B : (k + 1) * LOCAL_B]
                    rs = k * H
                    grp_ps = pooled_E if k < ka else pooled_L
                    first = k == 0 or k == ka
                    last = k == ka - 1 or k == n_slices - 1
                    for s in range(3):
                        mm = nc.tensor.matmul(
                            grp_ps[32 * s : 32 * (s + 1), :256],
                            lhsT,
                            hs_sb[:, rs + 256 * s : rs + 256 * (s + 1)],
                            start=first, stop=last,
                        )
                        if last:
                            mm.then_inc(s_poolE if k < ka else s_poolL, 1)

                # LATE head (tail)
                pe_transposes("L")
                pe_dense("L")
                pe_cls("L")

            @block.vector
            def _(vector):
                # EARLY mean scale: one wide op across all three strips.
                vector.wait_ge(s_smalls, 48)
                vector.wait_ge(s_poolE, 3)
                nc.vector.tensor_scalar_mul(
                    pooled_sb[0:96, 0:256],
                    pooled_E[0:96, :256],
                    smalls_t[0:96, 6:7],
                ).then_inc(s_scE, 1)
                for c in (0, 2, 4):
                    vector.wait_ge(s_trE, c + 1)
                    nc.vector.tensor_copy(
                        pooledT_sb[:, c * NE : (c + 1) * NE],
                        tp[c % 2][:, :NE],
                    ).then_inc(s_ptcE, 1)
                vector.wait_ge(s_clsE, 1)
                nc.vector.tensor_add(
                    logits_sb[0:NE, :], lps[0:NE, :T_OUT], clsb_t[0:NE, :]
                ).then_inc(s_logE, 1)
                # LATE
                vector.wait_ge(s_poolL, 3)
                nc.vector.tensor_scalar_mul(
                    pooled_sb[0:96, 256:512],
                    pooled_L[0:96, :256],
                    smalls_t[0:96, 7:8],
                ).then_inc(s_scL, 1)
                for c in (0, 2, 4):
                    vector.wait_ge(s_trL, c + 1)
                    nc.vector.tensor_copy(
                        pooledT_sb[:, 144 + c * NL : 144 + (c + 1) * NL],
                        tp[c % 2][:, :NL],
                    ).then_inc(s_ptcL, 1)
                vector.wait_ge(s_clsL, 1)
                # clsb rows are all identical (broadcast cls_b), so rows
                # 32:40 are partition-aligned with the LATE lps region.
                nc.vector.tensor_add(
                    logits_sb[32 : 32 + NL, :],
                    lps[32 : 32 + NL, :T_OUT],
                    clsb_t[32 : 32 + NL, :],
                ).then_inc(s_logL, 1)

            @block.scalar
            def _(scalar):
                # ACT HWDGE queue: member + consts + head weights.
                scalar.dma_start(out=member_t[:], in_=member_d[:]).then_inc(
                    s_member, 16
                )
                scalar.dma_start(out=smalls_t[:], in_=smalls_d[:]).then_inc(
                    s_smalls, 16
                )
                scalar.dma_start(out=identd_t[:], in_=identd_d[:]).then_inc(
                    s_smalls, 16
                )
                scalar.dma_start(out=clsb_t[:], in_=clsb_d[:]).then_inc(
                    s_smalls, 16
                )
                scalar.dma_start(
                    out=dwT_t[:, : 3 * H], in_=dwT_d[:, : 3 * H]
                ).then_inc(s_dwTa, 16)
                scalar.dma_start(
                    out=dwT_t[:, 3 * H :], in_=dwT_d[:, 3 * H :]
                ).then_inc(s_dwTb, 16)
                scalar.dma_start(out=cwT_t[:], in_=cwT_d[:]).then_inc(s_cwT, 16)
                # Dummy tanh: pulls the lazy ACT_TABLE_LOAD (~1.3us) forward.
                nc.scalar.activation(
                    scratch_sb[:, 0:1], warm_sb[:, 0:1],
                    mybir.ActivationFunctionType.Tanh,
                )

                def copies_tanh(grp):
                    s_tr = s_trE if grp == "E" else s_trL
                    for c in (1, 3, 5):
                        scalar.wait_ge(s_tr, c + 1)
                        if grp == "E":
                            dst = pooledT_sb[:, c * NE : (c + 1) * NE]
                            w = NE
                        else:
                            dst = pooledT_sb[:, 144 + c * NL : 144 + (c + 1) * NL]
                            w = NL
                        nc.scalar.activation(
                            dst, tp[c % 2][:, :w],
                            mybir.ActivationFunctionType.Copy,
                        ).then_inc(s_ptcE if grp == "E" else s_ptcL, 1)
                    sc = (1.0 / 64.0) if dwt8 else 1.0
                    if grp == "E":
                        if bias0:
                            for half in range(2):
                                scalar.wait_ge(s_headE, 3 * (half + 1))
                                nc.scalar.activation(
                                    hT_sb[:, 72 * half : 72 * (half + 1)],
                                    hps[half][:, : 3 * NE],
                                    mybir.ActivationFunctionType.Tanh,
                                    scale=sc,
                                ).then_inc(s_tanhE, 1)
                        else:
                            for jg in range(6):
                                scalar.wait_ge(s_headE, jg + 1)
                                nc.scalar.activation(
                                    hT_sb[:, jg * NE : (jg + 1) * NE],
                                    hps[jg // 3][
                                        :, (jg % 3) * NE : (jg % 3 + 1) * NE
                                    ],
                                    mybir.ActivationFunctionType.Tanh,
                                    bias=db6_ap[:, jg : jg + 1],
                                    scale=sc,
                                ).then_inc(s_tanhE, 1)
                    else:
                        if bias0:
                            scalar.wait_ge(s_headL, 6)
                            nc.scalar.activation(
                                hT_sb[:, 144:192],
                                hps[0][:, 96:144],
                                mybir.ActivationFunctionType.Tanh,
                                scale=sc,
                            ).then_inc(s_tanhL, 1)
                        else:
                            for jg in range(6):
                                scalar.wait_ge(s_headL, jg + 1)
                                nc.scalar.activation(
                                    hT_sb[:, 144 + jg * NL : 144 + (jg + 1) * NL],
                                    hps[0][:, 96 + jg * NL : 96 + (jg + 1) * NL],
                                    mybir.ActivationFunctionType.Tanh,
                                    bias=db6_ap[:, jg : jg + 1],
                                    scale=sc,
                                ).then_inc(s_tanhL, 1)

                copies_tanh("E")
                copies_tanh("L")

    nc.compile()
    return nc


def _ef_quantize(packed: np.ndarray, n_slices: int) -> np.ndarray:
    """Error-feedback quantization to fp8 e3m4, carried along the partition
    axis within each 128-row slice so each sample's device-side sum error
    collapses to its few chain-boundary carries."""
    arr = packed.reshape(n_slices, 128, H)
    q8 = np.empty((n_slices, 128, H), NP_F8)
    c = np.zeros((n_slices, H), np.float32)
    for p in range(128):
        y = arr[:, p, :] + c
        q = y.astype(NP_F8)
        c = y - q.astype(np.float32)
        q8[:, p, :] = q
    return q8


def _split_groups(lens_c: np.ndarray):
    """Pick NL samples for the LATE group with total rows <= L_SLICES*128,
    as close to it as possible (their stream time hides the EARLY head)."""
    cap = L_SLICES * 128
    order = np.argsort(lens_c, kind="stable")          # ascending
    late = list(order[:NL])                            # start: NL smallest
    rest = list(order[NL:])
    lsum = int(lens_c[late].sum())
    improved = True
    while improved:
        improved = False
        for i in range(len(late)):
            for j in range(len(rest)):
                d = int(lens_c[rest[j]]) - int(lens_c[late[i]])
                if d > 0 and lsum + d <= cap:
                    late[i], rest[j] = rest[j], late[i]
                    lsum += d
                    improved = True
                    break
            if improved:
                break
    early = [i for i in range(len(lens_c)) if i not in set(late)]
    return early, late


def kernel(hidden_states, pivot_len_list, dense_w, dense_b, cls_w, cls_b):
    global last_results
    hs = np.ascontiguousarray(np.asarray(hidden_states, dtype=np.float32))
    lens = np.asarray(pivot_len_list).astype(np.int64)
    dense_w = np.asarray(dense_w, dtype=np.float32)
    dense_b = np.asarray(dense_b, dtype=np.float32)
    cls_w = np.asarray(cls_w, dtype=np.float32)
    cls_b = np.asarray(cls_b, dtype=np.float32)
    assert hs.shape == (B, S, H), hs.shape
    assert lens.shape == (B,), lens.shape

    # ---- assign samples to cores: greedy LPT with a hard 32-per-core cap
    order = np.argsort(-lens, kind="stable")
    core_samples = [[] for _ in range(N_CORES)]
    load = np.zeros(N_CORES, dtype=np.int64)
    for b in order:
        open_cores = [c for c in range(N_CORES) if len(core_samples[c]) < LOCAL_B]
        c = min(open_cores, key=lambda c: load[c])
        core_samples[c].append(int(b))
        load[c] += int(lens[b])

    # ---- EARLY/LATE split per core; shared Ka across cores
    core_groups = []
    ka_need = 2
    for c in range(N_CORES):
        lens_c = lens[core_samples[c]]
        early, late = _split_groups(lens_c)
        rows_e = int(lens_c[early].sum())
        ka_need = max(ka_need, -(-rows_e // 128))
        core_groups.append((early, late))
    n_slices = ka_need + L_SLICES
    ka = ka_need

    bias0 = int(np.all(dense_b == 0.0))
    key = (n_slices, DWT8, bias0)
    if key not in _cache:
        _cache[key] = _build_program(n_slices, DWT8, bias0)
    nc = _cache[key]

    # ---- shared (replicated) head tensors
    dwT_host = np.empty((128, 6 * H), np.float32)
    for jg in range(6):
        for cc in range(6):
            dwT_host[:, jg * H + cc * 128 : jg * H + (cc + 1) * 128] = dense_w[
                jg * 128 : (jg + 1) * 128, cc * 128 : (cc + 1) * 128
            ].T
    cwT_host = np.empty((128, 6 * T_OUT), np.float32)
    for jg in range(6):
        cwT_host[:, jg * T_OUT : (jg + 1) * T_OUT] = cls_w[
            :, jg * 128 : (jg + 1) * 128
        ].T
    smalls_base = np.zeros((128, 8), np.float32)
    smalls_base[:, 0:6] = dense_b.reshape(6, 128).T
    clsb_host = np.ascontiguousarray(
        np.broadcast_to(cls_b, (40, T_OUT)).astype(np.float32)
    )
    identd_host = np.zeros((96, LOCAL_B), np.float16)
    for s in range(3):
        identd_host[32 * s : 32 * (s + 1)] = np.eye(LOCAL_B, dtype=np.float16)

    # ---- per-core packing: EARLY rows pad to ka slices, then LATE rows
    hs2 = hs.reshape(B * S, H)
    NR = n_slices * 128
    in_maps = []
    for c in range(N_CORES):
        samples = core_samples[c]
        lens_c = lens[samples]
        early, late = core_groups[c]
        ordered = early + late                  # local order: EARLY then LATE
        samples_ord = [samples[i] for i in ordered]
        lens_ord = lens_c[ordered]

        packed = np.zeros((NR, H), np.float32)
        mem = np.zeros((128, n_slices * LOCAL_B), NP_F8)

        def put(rows_idx, local_bs, base):
            n = rows_idx.size
            packed[base : base + n] = hs2[rows_idx]
            j = base + np.arange(n)
            kq = j // 128
            p = j % 128
            mem[p, kq * LOCAL_B + local_bs] = NP_F8(1.0)

        idx_e = np.concatenate(
            [np.arange(samples[i] * S + 1, samples[i] * S + 1 + lens_c[i])
             for i in early]
        )
        lb_e = np.repeat(np.arange(NE), lens_c[early])
        put(idx_e, lb_e, 0)
        idx_l = np.concatenate(
            [np.arange(samples[i] * S + 1, samples[i] * S + 1 + lens_c[i])
             for i in late]
        )
        lb_l = np.repeat(np.arange(NL), lens_c[late])
        put(idx_l, lb_l, ka * 128)

        q8 = _ef_quantize(packed, n_slices)
        hsb_host = np.ascontiguousarray(
            q8.transpose(1, 0, 2).reshape(128, n_slices * H)
        )

        invl = 1.0 / lens_ord.astype(np.float32)
        smalls_host = smalls_base.copy()
        for s in range(3):
            smalls_host[32 * s : 32 * s + NE, 6] = invl[:NE]
            smalls_host[32 * s : 32 * s + NL, 7] = invl[NE:]

        in_maps.append(
            {
                "hsb": hsb_host,
                "member": mem,
                "dwT": (np.clip(dwT_host * 64.0, -15.5, 15.5).astype(NP_F8)
                        if DWT8 else dwT_host.astype(np.float16)),
                "cwT": cwT_host.astype(np.float16),
                "identd": identd_host,
                "smalls": smalls_host,
                "clsb": clsb_host,
            }
        )
        core_samples[c] = samples_ord

    trace = bool(os.environ.get("KERNEL_TRACE"))
    try:
        res = bass_utils.run_bass_kernel_spmd(
            nc, in_maps, list(range(N_CORES)), trace=trace
        )
    except Exception:
        # Transient NRT device errors clear on retry.
        res = bass_utils.run_bass_kernel_spmd(
            nc, in_maps, list(range(N_CORES)), trace=trace
        )
    last_results = res

    logits = np.zeros((B, T_OUT), np.float32)
    for c in range(N_CORES):
        logits[core_samples[c], :] = res.results[c]["out"].astype(np.float32)
    return logits


# revision 29
# speedup vs baseline: 1.1147x; 1.0536x over previous
"""Trainium2 Bass kernel: ragged mean-pool over [1, len_i] + Linear->tanh->Linear head.

Strategy (pure data parallel over batch, 8 NeuronCores):
  * Host: balance the 256 samples across 8 cores (32 each) by row count (LPT),
    gather the rows hidden_states[b, 1:len_b+1, :] into a dense per-core pack
    of 128-row "slices" (row j -> slice j//128, partition j%128), and encode
    rows in fp8 e3m4 with error-feedback quantization chained along the
    partition axis inside each slice -- the carries cancel in the device-side
    per-sample sums (~0.6% pooled error vs ~1.1% plain RNE).
  * Device: stream the packed rows partition-major (per-partition-contiguous
    DMA descriptors reach ~360+ GB/s) in chunks sized small->big->small so the
    pooling matmuls start early and chase the last bytes closely. Pooling =
    PE matmuls with the 0/1 membership matrix (fp8) as stationary operand,
    split into THREE concurrent 256-column streams on disjoint PE column
    strips (col_grps 0/1/2 -> partitions 32s..32s+32 of one PSUM bank).
    Mean scale (1/len) is folded into the PSUM->SBUF copies (DVE
    tensor_scalar / ScalarE activation-scale, split across both engines).
    Head: PE transposes -> dense (fp8 e3m4 weights x64, dequant via the
    tanh's scale) -> tanh -> classifier (fp16) fully on-chip; tanh table
    preloaded at kernel start; logits stored fp16 and upcast on host.
  * Host: scatter per-core logits [32, 96] back to the full [256, 96].

Compiled program depends only on (n_slices, chunk split) -- raggedness lives
in the data (packing + membership), so recompiles are rare.
"""

import os
from contextlib import ExitStack

import numpy as np
import ml_dtypes

import concourse.bass as bass
import concourse.mybir as mybir
from concourse import bacc, bass_utils

B, S, H, T_OUT = 256, 512, 768, 96
N_CORES = 8
LOCAL_B = B // N_CORES        # 32 samples per core
F32 = mybir.dt.float32
F16 = mybir.dt.float16
F8 = mybir.dt.float8e3       # e3m4: 1-3-4, max 15.5, ~1.1% RMS quant err
NP_F8 = ml_dtypes.float8_e3m4

# COLT=2: column-tile the a-half pooling matmuls across two PE strips.
COLT = int(os.environ.get("KERNEL_COLT", "2"))
# DWT8=1: dense weights quantized to fp8 e3m4 (x64 pre-scale, 1/64 folded
# into the tanh's scale) and fed to a mixed fp8xfp16 dense matmul.
DWT8 = int(os.environ.get("KERNEL_DWT8", "1"))
# MEMBITS=1: membership matrix bit-packed 8x in DRAM, unpacked to fp8 on the
# DVE during the stream (bit b of byte i -> member column b*NB8+i).
# Default OFF: the uint8 shift/and unpack hung on hardware (engine fault).
MEMBITS = int(os.environ.get("KERNEL_MEMBITS", "0"))
MODE = f"f8e3-colt{COLT}-dwt8{DWT8}"  # informational (test.py prints it)

_cache: dict = {}
last_results = None  # BassKernelResults of the most recent run (for test.py)


def _chunk_split(n_slices: int) -> tuple:
    """Split n_slices into DMA chunks. The PE consumes a chunk only once the
    whole chunk has landed, so chunks shrink toward the end of the stream
    ([8, 4, 2, 2] tail) -- the pooling matmuls chase the last bytes closely
    instead of serializing a big chunk's matmul burst after the DMA."""
    head = [8]
    tail = [4, 3, 2, 2, 1, 1]
    if n_slices < sum(head) + sum(tail) + 6:
        out = []
        left = n_slices
        for c in [4] * (n_slices // 4) + [n_slices % 4]:
            if c:
                out.append(c)
        return tuple(out)
    rest = n_slices - sum(head) - sum(tail)
    n_big = max(1, -(-rest // 6))
    base, extra = divmod(rest, n_big)
    return tuple(
        head + [base + (1 if i < extra else 0) for i in range(n_big)] + tail
    )


def _build_program(n_slices: int, colt: int, dwt8: int = 0, membits: int = 0,
                   bias0: int = 0) -> bass.Bass:
    chunks = _chunk_split(n_slices)
    n_chunks = len(chunks)
    chunk_start = np.cumsum([0] + list(chunks))[:-1]
    start_to_chunk = {int(s): i for i, s in enumerate(chunk_start)}

    # No collectives -> no partition id; skipping it drops 5 per-engine
    # TENSOR_LOADs (~2us) from the launch preamble.
    nc = bacc.Bacc(enable_partition_id=False, monotonic_sem_count=0)

    hsb_d = nc.declare_dram_parameter("hsb", [128, n_slices * H], F8, isOutput=False)
    NB8 = -(-(n_slices * LOCAL_B) // 8)
    if membits:
        member_d = nc.declare_dram_parameter(
            "membits", [128, NB8], mybir.dt.uint8, isOutput=False
        )
    else:
        member_d = nc.declare_dram_parameter(
            "member", [128, n_slices * LOCAL_B], F8, isOutput=False
        )
    # dwT jg-major: cols [jg*H + c*128 + j] = dense_w[jg*128+j, c*128+h]^T --
    # contiguous halves so the dense layer can start after half the transfer.
    WDT = F8 if dwt8 else F16
    dwT_d = nc.declare_dram_parameter("dwT", [128, 6 * H], WDT, isOutput=False)
    cwT_d = nc.declare_dram_parameter("cwT", [128, 6 * T_OUT], F16, isOutput=False)
    # identd = diag(1/len) fp16: the transpose's "identity" operand, folding
    # the per-sample mean scale into the PE transposes for free.
    identd_d = nc.declare_dram_parameter("identd", [96, LOCAL_B], F16, isOutput=False)
    smalls_d = nc.declare_dram_parameter("smalls", [128, 8], F32, isOutput=False)
    clsb_d = nc.declare_dram_parameter("clsb", [LOCAL_B, T_OUT], F32, isOutput=False)
    # fp16 store: logits are <1 in magnitude, fp16 rounding ~5e-5 relative;
    # halves the store's data phase. Host upcasts to f32.
    out_d = nc.declare_dram_parameter("out", [LOCAL_B, T_OUT], F16, isOutput=True)

    with ExitStack() as ctx:
        hs_sb = ctx.enter_context(nc.sbuf_tensor([128, n_slices * H], F8))
        member_t = ctx.enter_context(nc.sbuf_tensor([128, n_slices * LOCAL_B], F8))
        if membits:
            membits_t = ctx.enter_context(
                nc.sbuf_tensor([128, NB8], mybir.dt.uint8)
            )
            mbit_t = ctx.enter_context(
                nc.sbuf_tensor([128, NB8], mybir.dt.uint8)
            )
        dwT_t = ctx.enter_context(nc.sbuf_tensor([128, 6 * H], WDT))
        cwT_t = ctx.enter_context(nc.sbuf_tensor([128, 6 * T_OUT], F16))
        identd_t = ctx.enter_context(nc.sbuf_tensor([96, LOCAL_B], F16))
        smalls_t = ctx.enter_context(nc.sbuf_tensor([128, 8], F32))
        clsb_t = ctx.enter_context(nc.sbuf_tensor([LOCAL_B, T_OUT], F32))
        # pooled_sb [96, 256]: strip s (hidden cols 256s:256s+256) lives at
        # partitions 32s:32s+32 -- one wide DVE scale op covers all strips.
        pooled_sb = ctx.enter_context(nc.sbuf_tensor([96, 256], F16))
        pooledT_sb = ctx.enter_context(nc.sbuf_tensor([128, 6 * LOCAL_B], F16))
        hT_sb = ctx.enter_context(nc.sbuf_tensor([128, 6 * LOCAL_B], F16))
        logits_sb = ctx.enter_context(nc.sbuf_tensor([LOCAL_B, T_OUT], F16))
        warm_sb = ctx.enter_context(nc.sbuf_tensor([128, 512], F8))
        scratch_sb = ctx.enter_context(nc.sbuf_tensor([128, 8], F32))

        # PSUM budget (8 banks): pooled, tp0-2, hps0-2, lps.
        # The pooling runs as THREE concurrent 256-column matmul streams on
        # disjoint PE column strips: strip s handles hidden cols
        # [256s, 256s+256) and accumulates into partitions [32s, 32s+32) of
        # one PSUM bank (107ns/slice issue pitch instead of 320).
        pooled = ctx.enter_context(nc.psum_tensor([96, 512], F32))
        tp = [
            ctx.enter_context(nc.psum_tensor(f"tp{i}", [128, 512], F16))
            for i in range(3)
        ]
        hps = [
            ctx.enter_context(nc.psum_tensor(f"hps{i}", [128, 512], F32))
            for i in range(3)
        ]
        lps = ctx.enter_context(nc.psum_tensor([LOCAL_B, 512], F32))

        db6_ap = smalls_t[:, 0:6]

        s_member = nc.alloc_semaphore("s_member")
        s_member1 = nc.alloc_semaphore("s_member1")
        s_chunk = [nc.alloc_semaphore(f"s_chunk{i}") for i in range(n_chunks)]
        s_smalls = nc.alloc_semaphore("s_smalls")
        s_dwTa = nc.alloc_semaphore("s_dwTa")
        s_dwTb = nc.alloc_semaphore("s_dwTb")
        s_cwT = nc.alloc_semaphore("s_cwT")
        s_warm = nc.alloc_semaphore("s_warm")
        s_pool = nc.alloc_semaphore("s_pool")
        s_sc = nc.alloc_semaphore("s_sc")
        s_tr = nc.alloc_semaphore("s_tr")
        s_ptcopy = nc.alloc_semaphore("s_ptcopy")
        s_head = nc.alloc_semaphore("s_head")
        s_tanh = nc.alloc_semaphore("s_tanh")
        s_cls = nc.alloc_semaphore("s_cls")
        s_log = nc.alloc_semaphore("s_log")
        s_out = nc.alloc_semaphore("s_out")

        with nc.Block(no_gpsimd_drain=True) as block:

            @block.gpsimd
            def _(gpsimd):
                nc.gpsimd.memset(warm_sb[:], 0.0).then_inc(s_warm, 1)

            @block.sync
            def _(sync):
                # FIFO ring in consumption order. Head weights ride last: the
                # adds/transposes overlap their transfer, and only the dense
                # layer waits on them.
                # member front-slice first: the first pooling matmuls need
                # only the first chunk's member columns.
                for ci, (cs, cn) in enumerate(zip(chunk_start, chunks)):
                    sync.dma_start(
                        out=hs_sb[:, cs * H : (cs + cn) * H],
                        in_=hsb_d[:, cs * H : (cs + cn) * H],
                    ).then_inc(s_chunk[ci], 16)
                sync.wait_ge(s_log, 1)
                sync.dma_start(out=out_d[:], in_=logits_sb[:]).then_inc(s_out, 16)
                # Keep the completion wait: without it the framework
                # teardown's dma_reset can overlap the in-flight store
                # (observed a once-in-several-runs NaN output).
                if os.environ.get("KERNEL_STOREWAIT", "1") == "1":
                    sync.wait_ge(s_out, 16)

            @block.tensor
            def _(tensor):
                # Warmup fillers: ungate the PE clock (HAM) and keep it busy
                # until chunk 0 lands so the pooling burst runs at 2.4 GHz.
                tensor.wait_ge(s_warm, 1)
                for _ in range(14):
                    nc.tensor.matmul(
                        hps[0][:, :512], warm_sb[:, :128], warm_sb[:, :512],
                        start=True, stop=True,
                    )

                # Three concurrent matmul streams on disjoint PE column
                # strips (col_grps 0/1/2); they do NOT finish in program
                # order -- each stream's last matmul signals its own sem.
                tensor.wait_ge(s_member, 16)
                for k in range(n_slices):
                    ci = start_to_chunk.get(k)
                    if ci is not None:
                        if ci >= n_chunks - 3:
                            for _ in range(3):
                                nc.tensor.matmul(
                                    hps[0][:, :256], warm_sb[:, :128],
                                    warm_sb[:, :256], start=True, stop=True,
                                )
                        tensor.wait_ge(s_chunk[ci], 16)
                    lhsT = member_t[:, k * LOCAL_B : (k + 1) * LOCAL_B]
                    rs = k * H
                    for s in range(3):
                        mm = nc.tensor.matmul(
                            pooled[32 * s : 32 * (s + 1), :256],
                            lhsT,
                            hs_sb[:, rs + 256 * s : rs + 256 * (s + 1)],
                            start=(k == 0), stop=(k == n_slices - 1),
                        )
                        if k == n_slices - 1:
                            mm.then_inc(s_pool, 1)

                # transposes: pooledT[128h, 32b] per 128-col chunk.
                tensor.wait_ge(s_smalls, 48)
                tensor.wait_ge(s_sc, 1)
                for c in range(6):
                    s = c // 2
                    if c >= 3:
                        # tp copies run on DVE (even c) and ScalarE (odd c)
                        # and complete out of order: c-1 arrivals guarantee
                        # the copy of c-3 (this tp slot's reader) is done.
                        tensor.wait_ge(s_ptcopy, c - 1)
                    nc.tensor.transpose(
                        tp[c % 3][:, :LOCAL_B],
                        pooled_sb[
                            32 * s : 32 * (s + 1),
                            (c % 2) * 128 : (c % 2) * 128 + 128,
                        ],
                        identd_t[32 * s : 32 * (s + 1), :],
                    ).then_inc(s_tr, 1)

                # dense layer (fp16): hT[j, b] = tanh(db + dwT^T @ pooledT)
                tensor.wait_ge(s_ptcopy, 6)
                for jg in range(6):
                    if jg in (0, 3):  # halves gate at their first jg only
                        tensor.wait_ge(s_dwTa if jg < 3 else s_dwTb, 16)
                    if bias0:
                        # zero dense bias (per the spec): jg pairs share one
                        # hps bank (cols 0:32 / 32:64) so tanh can run as 3
                        # wide ACTIVATEs, and banks are never reused -- no
                        # tanh waits in the dense loop at all.
                        out_ap = hps[jg // 2][
                            :, (jg % 2) * LOCAL_B : (jg % 2 + 1) * LOCAL_B
                        ]
                    else:
                        if jg >= 3:
                            tensor.wait_ge(s_tanh, jg - 2)
                        out_ap = hps[jg % 3][:, :LOCAL_B]
                    for c in range(6):
                        mm = nc.tensor.matmul(
                            out_ap,
                            dwT_t[:, jg * H + c * 128 : jg * H + (c + 1) * 128],
                            pooledT_sb[:, c * LOCAL_B : (c + 1) * LOCAL_B],
                            start=(c == 0), stop=(c == 5),
                        )
                    mm.then_inc(s_head, 1)

                # classifier: logits[b, t] (hT chunk stationary -> batch-major)
                tensor.wait_ge(s_cwT, 16)
                for jg in range(6):
                    tensor.wait_ge(s_tanh, jg + 1)
                    mm = nc.tensor.matmul(
                        lps[:, :T_OUT],
                        hT_sb[:, jg * LOCAL_B : (jg + 1) * LOCAL_B],
                        cwT_t[:, jg * T_OUT : (jg + 1) * T_OUT],
                        start=(jg == 0), stop=(jg == 5),
                    )
                mm.then_inc(s_cls, 1)

            @block.vector
            def _(vector):
                if membits:
                    # unpack 8 bit-planes: (byte >> b) & 1 -> uint8 0/1 ->
                    # numeric convert to fp8 1.0; each plane fills a
                    # contiguous 268-col block of member_t.
                    vector.wait_ge(s_member, 16)
                    for b in range(8):
                        nc.vector.tensor_scalar(
                            mbit_t[:], membits_t[:], b, 1,
                            mybir.AluOpType.logical_shift_right,
                            mybir.AluOpType.bitwise_and,
                        )
                        nc.vector.tensor_copy(
                            member_t[:, b * NB8 : (b + 1) * NB8], mbit_t[:]
                        ).then_inc(s_unp, 1)
                # mean scale: one wide op across all three strips.
                vector.wait_ge(s_smalls, 48)
                vector.wait_ge(s_pool, 3)
                nc.vector.tensor_scalar_mul(
                    pooled_sb[0:96, 0:256],
                    pooled[0:96, :256],
                    smalls_t[0:96, 6:7],
                ).then_inc(s_sc, 1)
                for c in (0, 2, 4):
                    vector.wait_ge(s_tr, c + 1)
                    nc.vector.tensor_copy(
                        pooledT_sb[:, c * LOCAL_B : (c + 1) * LOCAL_B],
                        tp[c % 3][:, :LOCAL_B],
                    ).then_inc(s_ptcopy, 1)
                vector.wait_ge(s_cls, 1)
                nc.vector.tensor_add(
                    logits_sb[:], lps[:, :T_OUT], clsb_t[:]
                ).then_inc(s_log, 1)

            @block.scalar
            def _(scalar):
                scalar.dma_start(out=member_t[:], in_=member_d[:]).then_inc(
                    s_member, 16
                )
                scalar.dma_start(out=smalls_t[:], in_=smalls_d[:]).then_inc(
                    s_smalls, 16
                )
                scalar.dma_start(out=identd_t[:], in_=identd_d[:]).then_inc(
                    s_smalls, 16
                )
                scalar.dma_start(out=clsb_t[:], in_=clsb_d[:]).then_inc(
                    s_smalls, 16
                )
                scalar.dma_start(
                    out=dwT_t[:, : 3 * H], in_=dwT_d[:, : 3 * H]
                ).then_inc(s_dwTa, 16)
                scalar.dma_start(
                    out=dwT_t[:, 3 * H :], in_=dwT_d[:, 3 * H :]
                ).then_inc(s_dwTb, 16)
                scalar.dma_start(out=cwT_t[:], in_=cwT_d[:]).then_inc(s_cwT, 16)
                # Dummy tanh: pulls the lazy ACT_TABLE_LOAD (~1.3us) off the
                # critical path to kernel start.
                nc.scalar.activation(
                    scratch_sb[:, 0:1], warm_sb[:, 0:1],
                    mybir.ActivationFunctionType.Tanh,
                )
                # odd tp->pooledT copies ride ScalarE, halving the DVE-serial
                # copy stretch that gates the dense layer.
                for c in (1, 3, 5):
                    scalar.wait_ge(s_tr, c + 1)
                    nc.scalar.activation(
                        pooledT_sb[:, c * LOCAL_B : (c + 1) * LOCAL_B],
                        tp[c % 3][:, :LOCAL_B],
                        mybir.ActivationFunctionType.Copy,
                    ).then_inc(s_ptcopy, 1)
                if bias0:
                    for p in range(3):
                        scalar.wait_ge(s_head, 2 * p + 2)
                        nc.scalar.activation(
                            hT_sb[:, 2 * p * LOCAL_B : (2 * p + 2) * LOCAL_B],
                            hps[p][:, : 2 * LOCAL_B],
                            mybir.ActivationFunctionType.Tanh,
                            scale=(1.0 / 64.0) if dwt8 else 1.0,
                        ).then_inc(s_tanh, 2)
                else:
                    for jg in range(6):
                        scalar.wait_ge(s_head, jg + 1)
                        nc.scalar.activation(
                            hT_sb[:, jg * LOCAL_B : (jg + 1) * LOCAL_B],
                            hps[jg % 3][:, :LOCAL_B],
                            mybir.ActivationFunctionType.Tanh,
                            bias=db6_ap[:, jg : jg + 1],
                            scale=(1.0 / 64.0) if dwt8 else 1.0,
                        ).then_inc(s_tanh, 1)

    nc.compile()
    return nc


def _ef_quantize(packed: np.ndarray, n_slices: int) -> np.ndarray:
    """Error-feedback quantization to fp8 e3m4, carried along the partition
    axis within each 128-row slice (= packed row order, sample-major), so each
    sample's device-side sum error collapses to its few chain-boundary
    carries."""
    arr = packed.reshape(n_slices, 128, H)
    q8 = np.empty((n_slices, 128, H), NP_F8)
    c = np.zeros((n_slices, H), np.float32)
    for p in range(128):
        y = arr[:, p, :] + c
        q = y.astype(NP_F8)
        c = y - q.astype(np.float32)
        q8[:, p, :] = q
    return q8


def kernel(hidden_states, pivot_len_list, dense_w, dense_b, cls_w, cls_b):
    global last_results
    hs = np.ascontiguousarray(np.asarray(hidden_states, dtype=np.float32))
    lens = np.asarray(pivot_len_list).astype(np.int64)
    dense_w = np.asarray(dense_w, dtype=np.float32)
    dense_b = np.asarray(dense_b, dtype=np.float32)
    cls_w = np.asarray(cls_w, dtype=np.float32)
    cls_b = np.asarray(cls_b, dtype=np.float32)
    assert hs.shape == (B, S, H), hs.shape
    assert lens.shape == (B,), lens.shape

    # ---- assign samples to cores: greedy LPT with a hard 32-per-core cap
    order = np.argsort(-lens, kind="stable")
    core_samples = [[] for _ in range(N_CORES)]
    load = np.zeros(N_CORES, dtype=np.int64)
    for b in order:
        open_cores = [c for c in range(N_CORES) if len(core_samples[c]) < LOCAL_B]
        c = min(open_cores, key=lambda c: load[c])
        core_samples[c].append(int(b))
        load[c] += int(lens[b])
    n_slices = max(2, -(-int(load.max()) // 128))

    bias0 = int(np.all(dense_b == 0.0))
    key = (n_slices, COLT, DWT8, MEMBITS, bias0)
    if key not in _cache:
        _cache[key] = _build_program(n_slices, COLT, DWT8, MEMBITS, bias0)
    nc = _cache[key]

    # ---- shared (replicated) head tensors
    dwT_host = np.empty((128, 6 * H), np.float32)
    for jg in range(6):
        for c in range(6):
            dwT_host[:, jg * H + c * 128 : jg * H + (c + 1) * 128] = dense_w[
                jg * 128 : (jg + 1) * 128, c * 128 : (c + 1) * 128
            ].T
    cwT_host = np.empty((128, 6 * T_OUT), np.float32)
    for jg in range(6):
        cwT_host[:, jg * T_OUT : (jg + 1) * T_OUT] = cls_w[
            :, jg * 128 : (jg + 1) * 128
        ].T
    smalls_base = np.zeros((128, 8), np.float32)
    smalls_base[:, 0:6] = dense_b.reshape(6, 128).T
    clsb_host = np.ascontiguousarray(
        np.broadcast_to(cls_b, (LOCAL_B, T_OUT)).astype(np.float32)
    )
    identd_host = np.zeros((96, LOCAL_B), np.float16)
    for s in range(3):
        identd_host[32 * s : 32 * (s + 1)] = np.eye(LOCAL_B, dtype=np.float16)

    # ---- per-core packing
    hs2 = hs.reshape(B * S, H)
    NR = n_slices * 128
    in_maps = []
    for c in range(N_CORES):
        samples = core_samples[c]
        lens_c = lens[samples]
        idx = np.concatenate(
            [np.arange(b * S + 1, b * S + 1 + lens[b]) for b in samples]
        )
        n = idx.size
        packed = np.zeros((NR, H), np.float32)
        packed[:n] = hs2[idx]
        q8 = _ef_quantize(packed, n_slices)
        hsb_host = np.ascontiguousarray(
            q8.transpose(1, 0, 2).reshape(128, n_slices * H)
        )

        j = np.arange(n)
        kq = j // 128
        p = j % 128
        local_b = np.repeat(np.arange(LOCAL_B), lens_c)
        if MEMBITS:
            NB8 = -(-(n_slices * LOCAL_B) // 8)
            memb = np.zeros((128, n_slices * LOCAL_B), np.uint8)
            memb[p, kq * LOCAL_B + local_b] = 1
            pad = 8 * NB8 - n_slices * LOCAL_B
            if pad:
                memb = np.concatenate(
                    [memb, np.zeros((128, pad), np.uint8)], axis=1
                )
            # column j -> bit j//NB8 of byte j%NB8
            mem = np.zeros((128, NB8), np.uint8)
            for b in range(8):
                mem |= memb[:, b * NB8 : (b + 1) * NB8] << b
        else:
            mem = np.zeros((128, n_slices * LOCAL_B), NP_F8)
            mem[p, kq * LOCAL_B + local_b] = NP_F8(1.0)

        invl = 1.0 / lens_c.astype(np.float32)
        smalls_host = smalls_base.copy()
        for s in range(3):                 # per-strip scale (partitions 32s+)
            smalls_host[32 * s : 32 * (s + 1), 6] = invl

        in_maps.append(
            {
                "hsb": hsb_host,
                ("membits" if MEMBITS else "member"): mem,
                "dwT": (np.clip(dwT_host * 64.0, -15.5, 15.5).astype(NP_F8)
                        if DWT8 else dwT_host.astype(np.float16)),
                "cwT": cwT_host.astype(np.float16),
                "identd": identd_host,
                "smalls": smalls_host,
                "clsb": clsb_host,
            }
        )

    trace = bool(os.environ.get("KERNEL_TRACE"))
    try:
        res = bass_utils.run_bass_kernel_spmd(
            nc, in_maps, list(range(N_CORES)), trace=trace
        )
    except Exception:
        # Transient NRT device errors clear on retry.
        res = bass_utils.run_bass_kernel_spmd(
            nc, in_maps, list(range(N_CORES)), trace=trace
        )
    last_results = res

    logits = np.zeros((B, T_OUT), np.float32)
    for c in range(N_CORES):
        logits[core_samples[c], :] = res.results[c]["out"].astype(np.float32)
    return logits

